# revision 9
# baseline (speedup 1.0000x reference)
"""Trainium2 Bass kernel for nn_CAUM_82884278878389.

Shapes: B=16, NC=20, NH=50, D=512, HEADS=8, DH=64.
Key structure: c_rep/h_rep broadcasts mean the big GEMMs decompose into
small ones over 800 h-tokens (b,n) and 320 c-tokens (b,ci) instead of
16000 (b,ci,n) rows.  MHA has seq = b*nc, batch = (n, head); with
qkv = A[b,n] + B[b,ci] the scores decompose as
   scoresT[m,t] = (kB+bink)[:,m].q[:,t] + Ind[b'(m),:].(kA_n.q)[:,t]
one fused k=80 matmul per (head, m-tile, chunk); and
   att@v[dh,t]  = vB.exp + vA_n.(Ind^T.exp)
Sharding: data-parallel over batch b across 8 cores (2 batches each).
All activations live in [channel, token] layout; per-core token order is
t' = (n, b_loc, ci), so per-n slices are contiguous 40-column blocks.
"""

import sys

for _p in ("/opt/trn_rl_repo", "/root/.axon_site/_ro/pypackages"):
    if _p not in sys.path:
        sys.path.insert(0, _p)

import numpy as np
import ml_dtypes

import concourse.bass as bass
import concourse.bacc as bacc
import concourse.tile as tile
import concourse.mybir as mybir
from concourse import bass_utils

F32 = mybir.dt.float32
F32R = mybir.dt.float32r
BF16 = mybir.dt.bfloat16
AF = mybir.ActivationFunctionType
ALU = mybir.AluOpType
AX = mybir.AxisListType
BF = ml_dtypes.bfloat16

B, NCv, NH, D, HEADS, DH = 16, 20, 50, 512, 8, 64
NCORES = 8
BPC = B // NCORES            # 2 batches per core
TOK = BPC * NCv * NH         # 2000 own tokens
BCn = BPC * NCv              # 40 own (b,ci)
HTOK, CTOK = B * NH, B * NCv     # 800, 320
HOWN, COWN = BPC * NH, BPC * NCv  # 100, 40
CH = 400                     # token chunk = 10 n-groups
NCH = TOK // CH              # 5
MT = [128, 128, 64]          # m-tiles over 320 c-tokens

USE_F32R = True
RDT = F32R if USE_F32R else F32


def _r(ap):
    return ap


def _v(ap2d, dims, off=0):
    """Custom free-dim view of a (possibly partition-sliced) 2D AP."""
    return bass.AP(
        tensor=ap2d.tensor,
        offset=ap2d.offset + off,
        ap=[list(ap2d.ap[0])] + [[s, c] for s, c in dims],
    )


def _build():
    nc = bacc.Bacc("TRN2", target_bir_lowering=False, debug=False)

    def din(name, shape, dt=F32):
        return nc.dram_tensor(name, list(shape), dt, kind="ExternalInput").ap()

    D_ = {}
    D_["hT_all"] = din("hT_all", (512, HTOK), RDT)
    D_["hcat_own"] = din("hcat_own", (1536, HOWN), RDT)
    D_["cT_all"] = din("cT_all", (512, CTOK), RDT)
    D_["cT_own"] = din("cT_own", (512, COWN), RDT)
    D_["w2aT"] = din("w2aT", (512, 512), RDT)
    D_["w2bT"] = din("w2bT", (512, 512), RDT)
    D_["winqT"] = din("winqT", (512, 512), RDT)
    D_["winkT"] = din("winkT", (512, 512), RDT)
    D_["winvT"] = din("winvT", (512, 512), RDT)
    D_["binq"] = din("binq", (128, 4))
    D_["bink"] = din("bink", (128, 4))
    D_["binv"] = din("binv", (1, 512), RDT)
    D_["wfaT"] = din("wfaT", (1536, 512), RDT)
    D_["wfdT"] = din("wfdT", (512, 512), RDT)
    D_["b3c"] = din("b3c", (128, 4))
    D_["wcT"] = din("wcT", (512, 512), BF16)
    D_["wd1aT"] = din("wd1aT", (512, 512), RDT)
    D_["wd1bT"] = din("wd1bT", (512, 512), RDT)
    D_["bd1c"] = din("bd1c", (128, 4))
    D_["wd2T"] = din("wd2T", (512, 256), BF16)
    D_["bd2c"] = din("bd2c", (128, 2))
    D_["wd3T"] = din("wd3T", (256, 1), BF16)
    D_["bd3c"] = din("bd3c", (1, 1))
    D_["indM"] = din("indM", (16, CTOK), BF16)
    D_["indRT"] = din("indRT", (128, 48), BF16)
    D_["uT"] = nc.dram_tensor("uT", [512, BCn], F32, kind="ExternalOutput").ap()

    with tile.TileContext(nc) as tc, nc.allow_low_precision(
            reason="float32r storage for full-rate fp32 matmuls"):
        _emit(nc, tc, D_)
    nc.compile()
    return nc


def _emit(nc, tc, D_):
    con = tc.alloc_tile_pool(name="con", bufs=1)
    pp = tc.alloc_tile_pool(name="pp", bufs=4, space="PSUM")
    po2 = tc.alloc_tile_pool(name="po2", bufs=2, space="PSUM")

    def ptile(shape, tag="ps", pool=None):
        return (pool or pp).tile(shape, F32, tag="o2" if pool else "ps",
                                 name="ps_t")

    def ctile(shape, name, dt=F32, pool=None):
        return (pool or con).tile(shape, dt, tag=name, name=name)

    def load(pool, name, dram, P, Fs):
        ts_ = []
        n = (dram.shape[0] + P - 1) // P
        for k in range(n):
            p = min(P, dram.shape[0] - k * P)
            tl = pool.tile([p, Fs], dram.dtype, tag=f"{name}{k}", name=f"{name}{k}")
            nc.sync.dma_start(out=tl, in_=dram[k * P:k * P + p, :])
            ts_.append(tl)
        return ts_

    def mm_acc(ps, lhs_list, rhs_list, extra=None):
        n = len(lhs_list)
        tot = n + (1 if extra else 0)
        for i in range(n):
            nc.tensor.matmul(ps, _r(lhs_list[i]), _r(rhs_list[i]),
                             start=(i == 0), stop=(i == tot - 1))
        if extra:
            nc.tensor.matmul(ps, _r(extra[0]), _r(extra[1]),
                             start=False, stop=True)

    # ---------- constants / biases ----------
    ones16 = ctile([16, 1], "ones16", BF16)
    nc.vector.memset(ones16, 1.0)
    ones1f = ctile([1, 128], "ones1f")
    nc.vector.memset(ones1f, 1.0)
    ones1r = ctile([1, 128], "ones1r", RDT)
    nc.vector.tensor_copy(out=ones1r, in_=ones1f)
    binq_s = load(con, "binq_s", D_["binq"], 128, 4)[0]
    bink_s = load(con, "bink_s", D_["bink"], 128, 4)[0]
    binv_s = load(con, "binv_s", D_["binv"], 1, 512)[0]
    b3c_s = load(con, "b3c_s", D_["b3c"], 128, 4)[0]
    bd1c_s = load(con, "bd1c_s", D_["bd1c"], 128, 4)[0]
    bd2c_s = load(con, "bd2c_s", D_["bd2c"], 128, 2)[0]
    bd3c_s = load(con, "bd3c_s", D_["bd3c"], 1, 1)[0]
    indM_s = load(con, "indM_s", D_["indM"], 16, CTOK)[0]
    indRT_s = load(con, "indRT_s", D_["indRT"], 128, 48)[0]
    wc = load(con, "wc", D_["wcT"], 128, 512)
    wd1a = load(con, "wd1a", D_["wd1aT"], 128, 512)
    wd2 = load(con, "wd2", D_["wd2T"], 128, 256)
    wd3 = load(con, "wd3", D_["wd3T"], 128, 1)

    # ---------- P1: A2T/B2T (+ own) ----------
    pM = tc.alloc_tile_pool(name="pM", bufs=1)
    a2t = [ctile([128, HTOK], f"a2t{i}", RDT, pool=pM) for i in range(4)]
    b2t = [ctile([128, CTOK], f"b2t{i}", RDT, pool=pM) for i in range(4)]
    a2o = [ctile([128, HOWN], f"a2o{i}", RDT, pool=pM) for i in range(4)]
    b2o = [ctile([128, COWN], f"b2o{i}", RDT, pool=pM) for i in range(4)]
    cta = load(pM, "cta", D_["cT_all"], 128, CTOK)

    pA = tc.alloc_tile_pool(name="pA", bufs=1, side="right")
    ht = load(pA, "ht", D_["hT_all"], 128, HTOK)
    hto = load(pA, "hto", D_["hcat_own"], 128, HOWN)[4:8]
    cto_a = load(pA, "cto_a", D_["cT_own"], 128, COWN)
    w2a = load(pA, "w2a", D_["w2aT"], 128, 512)
    w2b = load(pA, "w2b", D_["w2bT"], 128, 512)

    for mt in range(4):
        for hf in range(2):
            ps = ptile([128, 400])
            mm_acc(ps, [w2b[k][:, mt * 128:(mt + 1) * 128] for k in range(4)],
                   [ht[k][:, hf * 400:(hf + 1) * 400] for k in range(4)])
            nc.scalar.copy(out=a2t[mt][:, hf * 400:(hf + 1) * 400], in_=ps)
        ps = ptile([128, CTOK])
        mm_acc(ps, [w2a[k][:, mt * 128:(mt + 1) * 128] for k in range(4)],
               [cta[k] for k in range(4)])
        nc.scalar.copy(out=b2t[mt], in_=ps)
        ps = ptile([128, HOWN])
        mm_acc(ps, [w2b[k][:, mt * 128:(mt + 1) * 128] for k in range(4)],
               [hto[k] for k in range(4)])
        nc.scalar.copy(out=a2o[mt], in_=ps)
        ps = ptile([128, COWN])
        mm_acc(ps, [w2a[k][:, mt * 128:(mt + 1) * 128] for k in range(4)],
               [cto_a[k] for k in range(4)])
        nc.scalar.copy(out=b2o[mt], in_=ps)
    pA.release()

    # ---------- P3: projections ----------
    kat = [ctile([64, HTOK], f"kat{h}", BF16) for h in range(HEADS)]
    ki = [ctile([80, CTOK], f"ki{h}", BF16) for h in range(HEADS)]
    vbt = [ctile([MT[i], 512], f"vbt{i}", BF16) for i in range(3)]
    qa = [ctile([128, HOWN], f"qa{i}") for i in range(4)]
    qb = [ctile([128, COWN], f"qb{i}") for i in range(4)]
    pV = tc.alloc_tile_pool(name="pV", bufs=2, side="right")
    vadr = tc.alloc_tile_pool(name="pD", bufs=1, space="DRAM").tile(
        [512, HTOK], BF16, name="vadr")

    pB = tc.alloc_tile_pool(name="pB", bufs=1, side="right")
    winq = load(pB, "winq", D_["winqT"], 128, 512)
    wink = load(pB, "wink", D_["winkT"], 128, 512)
    winv = load(pB, "winv", D_["winvT"], 128, 512)

    for ct in range(4):
        for hf in range(2):
            ps = ptile([128, 400])
            mm_acc(ps, [wink[k][:, ct * 128:(ct + 1) * 128] for k in range(4)],
                   [a2t[k][:, hf * 400:(hf + 1) * 400] for k in range(4)])
            for j in range(2):
                nc.vector.tensor_copy(
                    out=kat[ct * 2 + j][:, hf * 400:(hf + 1) * 400],
                    in_=ps[j * 64:(j + 1) * 64, :])
            ps = ptile([128, 400])
            mm_acc(ps, [winv[k][:, ct * 128:(ct + 1) * 128] for k in range(4)],
                   [a2t[k][:, hf * 400:(hf + 1) * 400] for k in range(4)])
            vstg = pV.tile([128, 400], BF16, tag="vstg", name="vstg")
            nc.vector.tensor_copy(out=vstg, in_=ps)
            nc.sync.dma_start(
                out=vadr[ct * 128:(ct + 1) * 128, hf * 400:(hf + 1) * 400],
                in_=vstg)
        # KI rows: kB + bink ; Ind
        ps = ptile([128, CTOK])
        mm_acc(ps, [wink[k][:, ct * 128:(ct + 1) * 128] for k in range(4)],
               [b2t[k] for k in range(4)])
        for j in range(2):
            h = ct * 2 + j
            nc.vector.tensor_scalar_add(
                out=ki[h][0:64, :], in0=ps[j * 64:(j + 1) * 64, :],
                scalar1=bink_s[j * 64:(j + 1) * 64, ct:ct + 1])
            nc.vector.tensor_copy(out=ki[h][64:80, :], in_=indM_s)
        # q (own)
        ps = ptile([128, HOWN])
        mm_acc(ps, [winq[k][:, ct * 128:(ct + 1) * 128] for k in range(4)],
               [a2o[k] for k in range(4)])
        nc.scalar.copy(out=qa[ct], in_=ps)
        ps = ptile([128, COWN])
        mm_acc(ps, [winq[k][:, ct * 128:(ct + 1) * 128] for k in range(4)],
               [b2o[k] for k in range(4)])
        nc.scalar.copy(out=qb[ct], in_=ps)
    # vB token-major (+ binv via rank-1 ones)
    for mt in range(3):
        p = MT[mt]
        ps = ptile([p, 512])
        mm_acc(ps, [b2t[k][:, mt * 128:mt * 128 + p] for k in range(4)],
               [winv[k] for k in range(4)],
               extra=(ones1r[:, 0:p], binv_s))
        nc.vector.tensor_copy(out=vbt[mt], in_=ps)
    pB.release()
    pM.release()

    # ---------- P4: own-shard c/cnn decomposed parts ----------
    a1w3 = [ctile([128, HOWN], f"a1w3{i}") for i in range(4)]
    b1w3 = [ctile([128, COWN], f"b1w3{i}") for i in range(4)]
    cwd1 = [ctile([128, COWN], f"cwd1{i}") for i in range(4)]
    pC = tc.alloc_tile_pool(name="pC", bufs=1, side="right")
    hcato = load(pC, "hcato", D_["hcat_own"], 128, HOWN)
    cto = load(pC, "cto", D_["cT_own"], 128, COWN)
    wfa = load(pC, "wfa", D_["wfaT"], 128, 512)
    wfd = load(pC, "wfd", D_["wfdT"], 128, 512)
    wd1b = load(pC, "wd1b", D_["wd1bT"], 128, 512)
    for mt in range(4):
        sl = slice(mt * 128, (mt + 1) * 128)
        ps = ptile([128, HOWN])
        mm_acc(ps, [wfa[k][:, sl] for k in range(12)],
               [hcato[k] for k in range(12)])
        nc.scalar.copy(out=a1w3[mt], in_=ps)
        ps = ptile([128, COWN])
        mm_acc(ps, [wfd[k][:, sl] for k in range(4)], [cto[k] for k in range(4)])
        nc.scalar.copy(out=b1w3[mt], in_=ps)
        ps = ptile([128, COWN])
        mm_acc(ps, [wd1b[k][:, sl] for k in range(4)], [cto[k] for k in range(4)])
        nc.scalar.copy(out=cwd1[mt], in_=ps)
    pC.release()

    # ---------- P5: attention ----------
    attn = [ctile([128, TOK], f"attn{hp}", BF16) for hp in range(4)]
    ap_ = tc.alloc_tile_pool(name="ap", bufs=2)
    qpool = tc.alloc_tile_pool(name="qp", bufs=3)
    for hp in range(4):
        heads = (2 * hp, 2 * hp + 1)
        qt2, vaq = {}, {}
        for h in heads:
            qt = qpool.tile([80, TOK], BF16, tag="qt2", name=f"qt2_{h}")
            qt2[h] = qt
            ct, j = h // 2, h % 2
            rsl = slice(j * 64, (j + 1) * 64)
            for bl in range(BPC):
                nc.vector.scalar_tensor_tensor(
                    out=_v(qt[0:64, :], [[BCn, NH], [1, NCv]], off=bl * NCv),
                    in0=_v(qa[ct][rsl, :], [[1, NH], [0, NCv]], off=bl * NH),
                    scalar=binq_s[rsl, ct:ct + 1],
                    in1=_v(qb[ct][rsl, :], [[0, NH], [1, NCv]], off=bl * NCv),
                    op0=ALU.add, op1=ALU.add)
            for cc in range(NCH):
                ps = ptile([16, CH], tag="t2")
                for i in range(10):
                    n = cc * 10 + i
                    nc.tensor.matmul(
                        ps[:, i * 40:(i + 1) * 40],
                        _v(kat[h], [[NH, 16]], off=n),
                        qt[0:64, n * 40:(n + 1) * 40],
                        start=True, stop=True)
                nc.vector.tensor_copy(out=qt[64:80, cc * CH:(cc + 1) * CH],
                                      in_=ps)
            vq = qpool.tile([16, DH, NH], BF16, tag="vaq", name=f"vaq{h}")
            vaq[h] = vq
            nc.gpsimd.dma_start(out=vq, in_=bass.AP(
                tensor=vadr.tensor, offset=vadr.offset + h * DH * HTOK,
                ap=[[NH, 16], [HTOK, DH], [1, NH]]))
        for cc in range(NCH):
            csl = slice(cc * CH, (cc + 1) * CH)
            expt, rsb, msb = {}, {}, {}
            for h in heads:
                for mt in range(3):
                    p = MT[mt]
                    ps = ptile([p, CH], tag="sc")
                    nc.tensor.matmul(ps, ki[h][:, mt * 128:mt * 128 + p],
                                     qt2[h][:, csl], start=True, stop=True)
                    et = ap_.tile([p, CH], BF16, tag=f"exp{mt}", bufs=4,
                                  name=f"exp{mt}")
                    expt[(h, mt)] = et
                    nc.scalar.activation(out=et, in_=ps, func=AF.Exp)
                ps = ptile([16, CH], tag="rps")
                for mt in range(3):
                    p = MT[mt]
                    nc.tensor.matmul(ps, indRT_s[0:p, mt * 16:(mt + 1) * 16],
                                     expt[(h, mt)], start=(mt == 0),
                                     stop=(mt == 2))
                rt = ap_.tile([16, CH], BF16, tag="rsb", bufs=4, name="rsb")
                rsb[h] = rt
                nc.scalar.copy(out=rt, in_=ps)
                ps = ptile([1, CH], tag="sps")
                nc.tensor.matmul(ps, ones16, rt, start=True, stop=True)
                ssb = ap_.tile([1, CH], RDT, tag="ssb", bufs=4, name="ssb")
                nc.scalar.copy(out=ssb, in_=ps)
                ps = ptile([64, CH], tag="mps")
                nc.tensor.matmul(ps, _r(ones1r[:, 0:64]), _r(ssb),
                                 start=True, stop=True)
                mtl = ap_.tile([64, CH], F32, tag="msb", bufs=4, name="msb")
                msb[h] = mtl
                nc.vector.reciprocal(out=mtl, in_=ps)
            ps = ptile([128, CH], tag="o2", pool=po2)
            for j, h in enumerate(heads):
                osl = slice(j * 64, (j + 1) * 64)
                for mt in range(3):
                    nc.tensor.matmul(
                        ps[osl, :], vbt[mt][:, h * DH:(h + 1) * DH],
                        expt[(h, mt)], start=(mt == 0), stop=False,
                        skip_group_check=True)
                for i in range(10):
                    n = cc * 10 + i
                    nc.tensor.matmul(
                        ps[osl, i * 40:(i + 1) * 40], vaq[h][:, :, n],
                        rsb[h][:, i * 40:(i + 1) * 40],
                        start=False, stop=(i == 9),
                        skip_group_check=True)
            for j, h in enumerate(heads):
                nc.vector.tensor_tensor(
                    out=attn[hp][j * 64:(j + 1) * 64, csl],
                    in0=ps[j * 64:(j + 1) * 64, :], in1=msb[h], op=ALU.mult)

    # ---------- P6/P7/P8 fused per chunk ----------
    upar = [ctile([128, NCH, BCn], f"upar{i}") for i in range(4)]
    ssump = ctile([1, NCH, BCn], "ssump")
    hpool = tc.alloc_tile_pool(name="hp", bufs=2)
    for cc in range(NCH):
        csl = slice(cc * CH, (cc + 1) * CH)
        hall, a1, a2 = [], [], []
        for mt in range(4):
            ps = ptile([128, CH], tag="hps")
            for k in range(4):
                nc.tensor.matmul(ps, wc[k][:, mt * 128:(mt + 1) * 128],
                                 attn[k][:, csl], start=(k == 0), stop=(k == 3))
            hl = hpool.tile([128, CH], RDT, tag=f"hall{mt}", name=f"hall{mt}")
            hall.append(hl)
            for bl in range(BPC):
                ov = _v(hl, [[BCn, 10], [1, NCv]], off=bl * NCv)
                nc.vector.scalar_tensor_tensor(
                    out=ov, in0=_v(ps, [[BCn, 10], [1, NCv]], off=bl * NCv),
                    scalar=b3c_s[:, mt:mt + 1],
                    in1=_v(a1w3[mt], [[1, 10], [0, NCv]],
                           off=cc * 10 + bl * NH),
                    op0=ALU.add, op1=ALU.add)
                nc.vector.tensor_tensor(
                    out=ov, in0=ov,
                    in1=_v(b1w3[mt], [[0, 10], [1, NCv]], off=bl * NCv),
                    op=ALU.add)
        for mt in range(4):
            ps = ptile([128, CH], tag="hps")
            mm_acc(ps, [wd1a[k][:, mt * 128:(mt + 1) * 128] for k in range(4)],
                   [hall[k] for k in range(4)])
            tmp = hpool.tile([128, CH], F32, tag="a1tmp", name="a1tmp")
            for bl in range(BPC):
                nc.vector.tensor_tensor(
                    out=_v(tmp, [[BCn, 10], [1, NCv]], off=bl * NCv),
                    in0=_v(ps, [[BCn, 10], [1, NCv]], off=bl * NCv),
                    in1=_v(cwd1[mt], [[0, 10], [1, NCv]], off=bl * NCv),
                    op=ALU.add)
            atl = hpool.tile([128, CH], BF16, tag=f"a1_{mt}", name=f"a1_{mt}")
            a1.append(atl)
            nc.scalar.activation(out=atl, in_=tmp, func=AF.Tanh,
                                 bias=bd1c_s[:, mt:mt + 1], scale=1.0)
        for mt in range(2):
            ps = ptile([128, CH], tag="hps")
            for k in range(4):
                nc.tensor.matmul(ps, wd2[k][:, mt * 128:(mt + 1) * 128],
                                 a1[k], start=(k == 0), stop=(k == 3))
            atl = hpool.tile([128, CH], BF16, tag=f"a2_{mt}", name=f"a2_{mt}")
            a2.append(atl)
            nc.scalar.activation(out=atl, in_=ps, func=AF.Tanh,
                                 bias=bd2c_s[:, mt:mt + 1], scale=1.0)
        ps = ptile([1, CH], tag="a3ps")
        for k in range(2):
            nc.tensor.matmul(ps, wd3[k], a2[k], start=(k == 0), stop=(k == 1))
        ea3 = hpool.tile([1, CH], RDT, tag="ea3", name="ea3")
        nc.scalar.activation(out=ea3, in_=ps, func=AF.Exp, bias=bd3c_s,
                             scale=1.0)
        nc.vector.tensor_reduce(
            out=ssump[:, cc, :], in_=_v(ea3, [[1, BCn], [BCn, 10]]),
            axis=AX.X, op=ALU.add)
        psw = ptile([128, CH], tag="wps")
        nc.tensor.matmul(psw, _r(ones1r), _r(ea3), start=True, stop=True)
        for mt in range(4):
            tmp = hpool.tile([128, CH], F32, tag="utmp", name="utmp")
            nc.vector.tensor_tensor(out=tmp, in0=hall[mt], in1=psw,
                                    op=ALU.mult)
            nc.vector.tensor_reduce(
                out=upar[mt][:, cc, :], in_=_v(tmp, [[1, BCn], [BCn, 10]]),
                axis=AX.X, op=ALU.add)

    ssum = ctile([1, BCn], "ssum")
    nc.vector.tensor_reduce(out=ssum, in_=_v(ssump, [[1, BCn], [BCn, NCH]]),
                            axis=AX.X, op=ALU.add)
    rw = ctile([1, BCn], "rw", RDT)
    nc.vector.reciprocal(out=rw, in_=ssum)
    psr = ptile([128, BCn], tag="rwb")
    nc.tensor.matmul(psr, _r(ones1r), _r(rw), start=True, stop=True)
    for mt in range(4):
        us = ctile([128, BCn], f"usb{mt}")
        nc.vector.tensor_reduce(
            out=us, in_=_v(upar[mt], [[1, BCn], [BCn, NCH]]),
            axis=AX.X, op=ALU.add)
        nc.vector.tensor_tensor(out=us, in0=us, in1=psr, op=ALU.mult)
        nc.sync.dma_start(out=D_["uT"][mt * 128:(mt + 1) * 128, :], in_=us)
    # release remaining pools in LIFO order per (space, side) stack
    hpool.release()
    qpool.release()
    ap_.release()
    con.release()
    pV.release()
    po2.release()
    pp.release()


def _host_prep(h, c, W1, b1, W2, b2, W3, b3, Wd1, bd1, Wd2, bd2, Wd3, bd3,
               Win, bin_, Wout, bout):
    f = np.float32
    ct = np.ascontiguousarray
    h = np.asarray(h, f); c = np.asarray(c, f)
    W1, b1, W2, b2, W3, b3 = (np.asarray(x, f) for x in (W1, b1, W2, b2, W3, b3))
    Wd1, bd1, Wd2, bd2, Wd3, bd3 = (np.asarray(x, f) for x in
                                    (Wd1, bd1, Wd2, bd2, Wd3, bd3))
    Win, bin_, Wout, bout = (np.asarray(x, f) for x in (Win, bin_, Wout, bout))

    W3a, W3b = W3[:, :512], W3[:, 512:]
    Wfa = W3a @ W1[:, :1536]
    Wfd = W3a @ W1[:, 1536:]
    b3p = b3 + W3b @ bout + W3a @ b1
    Winq, Wink, Winv = Win[:512], Win[512:1024], Win[1024:]
    binq = (bin_[:512] + Winq @ b2) / 8.0
    bink = bin_[512:1024] + Wink @ b2
    binv = bin_[1024:] + Winv @ b2
    Wc = W3b @ Wout

    hT = ct(h.transpose(2, 0, 1).reshape(512, HTOK))
    hlT = np.roll(h, 1, axis=1).transpose(2, 0, 1).reshape(512, HTOK)
    hrT = np.roll(h, -1, axis=1).transpose(2, 0, 1).reshape(512, HTOK)
    hcat_all = np.concatenate([hlT, hT, hrT], axis=0)
    cT_all = ct(c.transpose(2, 0, 1).reshape(512, CTOK))

    col = lambda v, k: ct(np.asarray(v, f).reshape(k, 128).T)
    indM = np.zeros((16, CTOK), BF)
    for m in range(CTOK):
        indM[m // NCv, m] = 1
    indRT = np.zeros((128, 3, 16), BF)
    for m in range(CTOK):
        indRT[m % 128, m // 128, m // NCv] = 1

    shared = dict(
        hT_all=hT, cT_all=cT_all,
        w2aT=ct(W2[:, :512].T), w2bT=ct(W2[:, 512:].T),
        winqT=ct(Winq.T / 8.0), winkT=ct(Wink.T), winvT=ct(Winv.T),
        binq=col(binq, 4), bink=col(bink, 4),
        binv=ct(binv.reshape(1, 512)),
        wfaT=ct(Wfa.T), wfdT=ct(Wfd.T), b3c=col(b3p, 4),
        wcT=ct(Wc.T.astype(BF)),
        wd1aT=ct(Wd1[:, :512].T), wd1bT=ct(Wd1[:, 512:].T),
        bd1c=col(bd1, 4),
        wd2T=ct(Wd2.T.astype(BF)), bd2c=col(bd2, 2),
        wd3T=ct(Wd3.T.astype(BF)),
        bd3c=np.asarray(bd3, f).reshape(1, 1),
        indM=indM, indRT=ct(indRT.reshape(128, 48)),
    )
    in_maps = []
    for r in range(NCORES):
        m = dict(shared)
        m["hcat_own"] = ct(hcat_all[:, 2 * r * NH:(2 * r + 2) * NH])
        m["cT_own"] = ct(cT_all[:, 2 * r * NCv:(2 * r + 2) * NCv])
        in_maps.append(m)
    return in_maps


_PROG = None


def _get_prog():
    global _PROG
    if _PROG is None:
        _PROG = _build()
    return _PROG


def kernel(**inputs):
    nc = _get_prog()
    in_maps = _host_prep(**inputs)
    res = bass_utils.run_bass_kernel_spmd(nc, in_maps,
                                          core_ids=list(range(NCORES)))
    u = np.empty((B, NCv, 512), np.float32)
    for r in range(NCORES):
        uT = res.results[r]["uT"]
        u[BPC * r:BPC * (r + 1)] = uT.T.reshape(BPC, NCv, 512)
    return u


# revision 10
# speedup vs baseline: 1.3774x; 1.3774x over previous
"""Trainium2 Bass kernel for nn_CAUM_82884278878389.

Shapes: B=16, NC=20, NH=50, D=512, HEADS=8, DH=64.
Key structure: c_rep/h_rep broadcasts mean the big GEMMs decompose into
small ones over 800 h-tokens (b,n) and 320 c-tokens (b,ci) instead of
16000 (b,ci,n) rows.  MHA has seq = b*nc, batch = (n, head); with
qkv = A[b,n] + B[b,ci] the scores decompose as
   scoresT[m,t] = (kB+bink)[:,m].q[:,t] + Ind[b'(m),:].(kA_n.q)[:,t]
one fused k=80 matmul per (head, m-tile, chunk); and
   att@v[dh,t]  = vB.exp + vA_n.(Ind^T.exp)
Sharding: data-parallel over batch b across 8 cores (2 batches each).
All activations live in [channel, token] layout; per-core token order is
t' = (n, b_loc, ci), so per-n slices are contiguous 40-column blocks.
"""

import sys

for _p in ("/opt/trn_rl_repo", "/root/.axon_site/_ro/pypackages"):
    if _p not in sys.path:
        sys.path.insert(0, _p)

import numpy as np
import ml_dtypes

import concourse.bass as bass
import concourse.bacc as bacc
import concourse.tile as tile
import concourse.mybir as mybir
from concourse import bass_utils

F32 = mybir.dt.float32
F32R = mybir.dt.float32r
BF16 = mybir.dt.bfloat16
AF = mybir.ActivationFunctionType
ALU = mybir.AluOpType
AX = mybir.AxisListType
BF = ml_dtypes.bfloat16

B, NCv, NH, D, HEADS, DH = 16, 20, 50, 512, 8, 64
NCORES = 8
BPC = B // NCORES            # 2 batches per core
TOK = BPC * NCv * NH         # 2000 own tokens
BCn = BPC * NCv              # 40 own (b,ci)
HTOK, CTOK = B * NH, B * NCv     # 800, 320
HOWN, COWN = BPC * NH, BPC * NCv  # 100, 40
CH = 400                     # token chunk = 10 n-groups
NCH = TOK // CH              # 5
MT = [128, 128, 64]          # m-tiles over 320 c-tokens

USE_F32R = True
RDT = F32R if USE_F32R else F32


def _r(ap):
    return ap


def _v(ap2d, dims, off=0):
    """Custom free-dim view of a (possibly partition-sliced) 2D AP."""
    return bass.AP(
        tensor=ap2d.tensor,
        offset=ap2d.offset + off,
        ap=[list(ap2d.ap[0])] + [[s, c] for s, c in dims],
    )


def _build():
    nc = bacc.Bacc("TRN2", target_bir_lowering=False, debug=False)

    def din(name, shape, dt=F32):
        return nc.dram_tensor(name, list(shape), dt, kind="ExternalInput").ap()

    D_ = {}
    D_["hT_all"] = din("hT_all", (512, HTOK), RDT)
    D_["hcat_own"] = din("hcat_own", (1536, HOWN), RDT)
    D_["cT_all"] = din("cT_all", (512, CTOK), RDT)
    D_["cT_own"] = din("cT_own", (512, COWN), RDT)
    D_["w2aT"] = din("w2aT", (512, 512), RDT)
    D_["w2bT"] = din("w2bT", (512, 512), RDT)
    D_["winqT"] = din("winqT", (512, 512), RDT)
    D_["winkT"] = din("winkT", (512, 512), RDT)
    D_["winvT"] = din("winvT", (512, 512), RDT)
    D_["binq"] = din("binq", (128, 4))
    D_["bink"] = din("bink", (128, 4))
    D_["binv"] = din("binv", (1, 512), RDT)
    D_["wfaT"] = din("wfaT", (1536, 512), RDT)
    D_["wfdT"] = din("wfdT", (512, 512), RDT)
    D_["b3c"] = din("b3c", (128, 4))
    D_["wcT"] = din("wcT", (512, 512), BF16)
    D_["wd1aT"] = din("wd1aT", (512, 512), RDT)
    D_["wd1bT"] = din("wd1bT", (512, 512), RDT)
    D_["bd1c"] = din("bd1c", (128, 4))
    D_["wd2T"] = din("wd2T", (512, 256), BF16)
    D_["bd2c"] = din("bd2c", (128, 2))
    D_["wd3T"] = din("wd3T", (256, 1), BF16)
    D_["bd3c"] = din("bd3c", (1, 1))
    D_["indM"] = din("indM", (16, CTOK), BF16)
    D_["indRT"] = din("indRT", (128, 48), BF16)
    D_["uT"] = nc.dram_tensor("uT", [512, BCn], F32, kind="ExternalOutput").ap()

    with tile.TileContext(nc) as tc, nc.allow_low_precision(
            reason="float32r storage for full-rate fp32 matmuls"):
        _emit(nc, tc, D_)
    nc.compile()
    return nc


def _emit(nc, tc, D_):
    con = tc.alloc_tile_pool(name="con", bufs=1)
    pp = tc.alloc_tile_pool(name="pp", bufs=2, space="PSUM")
    po2 = tc.alloc_tile_pool(name="po2", bufs=2, space="PSUM")

    def ptile(shape, tag="ps", pool=None):
        if pool is not None:
            return pool.tile(shape, F32, tag="o2", name="ps_o2")
        tag = tag if tag in ("sc", "aux") else "ps"
        return pp.tile(shape, F32, tag=tag, name=f"ps_{tag}")

    def ctile(shape, name, dt=F32, pool=None):
        return (pool or con).tile(shape, dt, tag=name, name=name)

    def load(pool, name, dram, P, Fs):
        ts_ = []
        n = (dram.shape[0] + P - 1) // P
        for k in range(n):
            p = min(P, dram.shape[0] - k * P)
            tl = pool.tile([p, Fs], dram.dtype, tag=f"{name}{k}", name=f"{name}{k}")
            nc.sync.dma_start(out=tl, in_=dram[k * P:k * P + p, :])
            ts_.append(tl)
        return ts_

    def mm_acc(ps, lhs_list, rhs_list, extra=None):
        n = len(lhs_list)
        tot = n + (1 if extra else 0)
        for i in range(n):
            nc.tensor.matmul(ps, _r(lhs_list[i]), _r(rhs_list[i]),
                             start=(i == 0), stop=(i == tot - 1))
        if extra:
            nc.tensor.matmul(ps, _r(extra[0]), _r(extra[1]),
                             start=False, stop=True)

    # ---------- constants / biases ----------
    ones16 = ctile([16, 1], "ones16", BF16)
    nc.vector.memset(ones16, 1.0)
    ones1f = ctile([1, 128], "ones1f")
    nc.vector.memset(ones1f, 1.0)
    ones1r = ctile([1, 128], "ones1r", RDT)
    nc.vector.tensor_copy(out=ones1r, in_=ones1f)
    binq_s = load(con, "binq_s", D_["binq"], 128, 4)[0]
    bink_s = load(con, "bink_s", D_["bink"], 128, 4)[0]
    binv_s = load(con, "binv_s", D_["binv"], 1, 512)[0]
    b3c_s = load(con, "b3c_s", D_["b3c"], 128, 4)[0]
    bd1c_s = load(con, "bd1c_s", D_["bd1c"], 128, 4)[0]
    bd2c_s = load(con, "bd2c_s", D_["bd2c"], 128, 2)[0]
    bd3c_s = load(con, "bd3c_s", D_["bd3c"], 1, 1)[0]
    indM_s = load(con, "indM_s", D_["indM"], 16, CTOK)[0]
    indRT_s = load(con, "indRT_s", D_["indRT"], 128, 48)[0]
    wc = load(con, "wc", D_["wcT"], 128, 512)
    wd1a = load(con, "wd1a", D_["wd1aT"], 128, 512)
    wd2 = load(con, "wd2", D_["wd2T"], 128, 256)
    wd3 = load(con, "wd3", D_["wd3T"], 128, 1)

    # ---------- P1: A2T/B2T (+ own) ----------
    pM = tc.alloc_tile_pool(name="pM", bufs=1)
    a2t = [ctile([128, HTOK], f"a2t{i}", RDT, pool=pM) for i in range(4)]
    b2t = [ctile([128, CTOK], f"b2t{i}", RDT, pool=pM) for i in range(4)]
    a2o = [ctile([128, HOWN], f"a2o{i}", RDT, pool=pM) for i in range(4)]
    b2o = [ctile([128, COWN], f"b2o{i}", RDT, pool=pM) for i in range(4)]
    cta = load(pM, "cta", D_["cT_all"], 128, CTOK)

    pA = tc.alloc_tile_pool(name="pA", bufs=1, side="right")
    ht = load(pA, "ht", D_["hT_all"], 128, HTOK)
    hto = load(pA, "hto", D_["hcat_own"], 128, HOWN)[4:8]
    cto_a = load(pA, "cto_a", D_["cT_own"], 128, COWN)
    w2a = load(pA, "w2a", D_["w2aT"], 128, 512)
    w2b = load(pA, "w2b", D_["w2bT"], 128, 512)

    for mt in range(4):
        for hf in range(2):
            ps = ptile([128, 400])
            mm_acc(ps, [w2b[k][:, mt * 128:(mt + 1) * 128] for k in range(4)],
                   [ht[k][:, hf * 400:(hf + 1) * 400] for k in range(4)])
            nc.scalar.copy(out=a2t[mt][:, hf * 400:(hf + 1) * 400], in_=ps)
        ps = ptile([128, CTOK])
        mm_acc(ps, [w2a[k][:, mt * 128:(mt + 1) * 128] for k in range(4)],
               [cta[k] for k in range(4)])
        nc.scalar.copy(out=b2t[mt], in_=ps)
        ps = ptile([128, HOWN])
        mm_acc(ps, [w2b[k][:, mt * 128:(mt + 1) * 128] for k in range(4)],
               [hto[k] for k in range(4)])
        nc.scalar.copy(out=a2o[mt], in_=ps)
        ps = ptile([128, COWN])
        mm_acc(ps, [w2a[k][:, mt * 128:(mt + 1) * 128] for k in range(4)],
               [cto_a[k] for k in range(4)])
        nc.scalar.copy(out=b2o[mt], in_=ps)
    pA.release()

    # ---------- P3: projections ----------
    kat = [ctile([64, HTOK], f"kat{h}", BF16) for h in range(HEADS)]
    ki = [ctile([80, CTOK], f"ki{h}", BF16) for h in range(HEADS)]
    vbt = [ctile([MT[i], 512], f"vbt{i}", BF16) for i in range(3)]
    qa = [ctile([128, HOWN], f"qa{i}") for i in range(4)]
    qb = [ctile([128, COWN], f"qb{i}") for i in range(4)]
    pV = tc.alloc_tile_pool(name="pV", bufs=2, side="right")
    vadr = tc.alloc_tile_pool(name="pD", bufs=1, space="DRAM").tile(
        [512, HTOK], BF16, name="vadr")

    pB = tc.alloc_tile_pool(name="pB", bufs=1, side="right")
    winq = load(pB, "winq", D_["winqT"], 128, 512)
    wink = load(pB, "wink", D_["winkT"], 128, 512)
    winv = load(pB, "winv", D_["winvT"], 128, 512)

    for ct in range(4):
        for hf in range(2):
            ps = ptile([128, 400])
            mm_acc(ps, [wink[k][:, ct * 128:(ct + 1) * 128] for k in range(4)],
                   [a2t[k][:, hf * 400:(hf + 1) * 400] for k in range(4)])
            for j in range(2):
                nc.vector.tensor_copy(
                    out=kat[ct * 2 + j][:, hf * 400:(hf + 1) * 400],
                    in_=ps[j * 64:(j + 1) * 64, :])
            ps = ptile([128, 400])
            mm_acc(ps, [winv[k][:, ct * 128:(ct + 1) * 128] for k in range(4)],
                   [a2t[k][:, hf * 400:(hf + 1) * 400] for k in range(4)])
            vstg = pV.tile([128, 400], BF16, tag="vstg", name="vstg")
            nc.vector.tensor_copy(out=vstg, in_=ps)
            nc.sync.dma_start(
                out=vadr[ct * 128:(ct + 1) * 128, hf * 400:(hf + 1) * 400],
                in_=vstg)
        # KI rows: kB + bink ; Ind
        ps = ptile([128, CTOK])
        mm_acc(ps, [wink[k][:, ct * 128:(ct + 1) * 128] for k in range(4)],
               [b2t[k] for k in range(4)])
        for j in range(2):
            h = ct * 2 + j
            nc.vector.tensor_scalar_add(
                out=ki[h][0:64, :], in0=ps[j * 64:(j + 1) * 64, :],
                scalar1=bink_s[j * 64:(j + 1) * 64, ct:ct + 1])
            nc.vector.tensor_copy(out=ki[h][64:80, :], in_=indM_s)
        # q (own)
        ps = ptile([128, HOWN])
        mm_acc(ps, [winq[k][:, ct * 128:(ct + 1) * 128] for k in range(4)],
               [a2o[k] for k in range(4)])
        nc.scalar.copy(out=qa[ct], in_=ps)
        ps = ptile([128, COWN])
        mm_acc(ps, [winq[k][:, ct * 128:(ct + 1) * 128] for k in range(4)],
               [b2o[k] for k in range(4)])
        nc.scalar.copy(out=qb[ct], in_=ps)
    # vB token-major (+ binv via rank-1 ones)
    for mt in range(3):
        p = MT[mt]
        ps = ptile([p, 512])
        mm_acc(ps, [b2t[k][:, mt * 128:mt * 128 + p] for k in range(4)],
               [winv[k] for k in range(4)],
               extra=(ones1r[:, 0:p], binv_s))
        nc.vector.tensor_copy(out=vbt[mt], in_=ps)
    pB.release()
    pM.release()

    # ---------- P4: own-shard c/cnn decomposed parts ----------
    a1w3 = [ctile([128, HOWN], f"a1w3{i}") for i in range(4)]
    b1w3 = [ctile([128, COWN], f"b1w3{i}") for i in range(4)]
    cwd1 = [ctile([128, COWN], f"cwd1{i}") for i in range(4)]
    pC = tc.alloc_tile_pool(name="pC", bufs=1, side="right")
    hcato = load(pC, "hcato", D_["hcat_own"], 128, HOWN)
    cto = load(pC, "cto", D_["cT_own"], 128, COWN)
    wfa = load(pC, "wfa", D_["wfaT"], 128, 512)
    wfd = load(pC, "wfd", D_["wfdT"], 128, 512)
    wd1b = load(pC, "wd1b", D_["wd1bT"], 128, 512)
    for mt in range(4):
        sl = slice(mt * 128, (mt + 1) * 128)
        ps = ptile([128, HOWN])
        mm_acc(ps, [wfa[k][:, sl] for k in range(12)],
               [hcato[k] for k in range(12)])
        nc.scalar.copy(out=a1w3[mt], in_=ps)
        ps = ptile([128, COWN])
        mm_acc(ps, [wfd[k][:, sl] for k in range(4)], [cto[k] for k in range(4)])
        nc.scalar.copy(out=b1w3[mt], in_=ps)
        ps = ptile([128, COWN])
        mm_acc(ps, [wd1b[k][:, sl] for k in range(4)], [cto[k] for k in range(4)])
        nc.scalar.copy(out=cwd1[mt], in_=ps)
    pC.release()

    # ---------- P5: attention ----------
    attn = [ctile([128, TOK], f"attn{hp}", BF16) for hp in range(4)]
    ap_ = tc.alloc_tile_pool(name="ap", bufs=2)
    qpool = tc.alloc_tile_pool(name="qp", bufs=3)
    for hp in range(4):
        heads = (2 * hp, 2 * hp + 1)
        qt2, vaq = {}, {}
        for h in heads:
            qt = qpool.tile([80, TOK], BF16, tag="qt2", name=f"qt2_{h}")
            qt2[h] = qt
            ct, j = h // 2, h % 2
            rsl = slice(j * 64, (j + 1) * 64)
            for bl in range(BPC):
                nc.vector.scalar_tensor_tensor(
                    out=_v(qt[0:64, :], [[BCn, NH], [1, NCv]], off=bl * NCv),
                    in0=_v(qa[ct][rsl, :], [[1, NH], [0, NCv]], off=bl * NH),
                    scalar=binq_s[rsl, ct:ct + 1],
                    in1=_v(qb[ct][rsl, :], [[0, NH], [1, NCv]], off=bl * NCv),
                    op0=ALU.add, op1=ALU.add)
            for cc in range(NCH):
                ps = ptile([16, CH], tag="aux")
                for i in range(10):
                    n = cc * 10 + i
                    nc.tensor.matmul(
                        ps[:, i * 40:(i + 1) * 40],
                        _v(kat[h], [[NH, 16]], off=n),
                        qt[0:64, n * 40:(n + 1) * 40],
                        start=True, stop=True)
                nc.scalar.copy(out=qt[64:80, cc * CH:(cc + 1) * CH], in_=ps)
            vq = qpool.tile([16, DH, NH], BF16, tag="vaq", name=f"vaq{h}")
            vaq[h] = vq
            nc.gpsimd.dma_start(out=vq, in_=bass.AP(
                tensor=vadr.tensor, offset=vadr.offset + h * DH * HTOK,
                ap=[[NH, 16], [HTOK, DH], [1, NH]]))
        expt, rsb, msb = {}, {}, {}
        SCH = 500
        for h in heads:
            for mt in range(3):
                p = MT[mt]
                expt[(h, mt)] = ap_.tile([p, TOK], BF16, tag=f"exp{mt}",
                                         bufs=2, name=f"exp{mt}")
            rsb[h] = ap_.tile([16, TOK], BF16, tag="rsb", bufs=2, name="rsb")
            msb[h] = ap_.tile([64, TOK], F32, tag="msb", bufs=2, name="msb")
        for h in heads:
            for cc in range(TOK // SCH):
                csl = slice(cc * SCH, (cc + 1) * SCH)
                for mt in range(3):
                    p = MT[mt]
                    ps = ptile([p, SCH], tag="sc")
                    nc.tensor.matmul(ps, ki[h][:, mt * 128:mt * 128 + p],
                                     qt2[h][:, csl], start=True, stop=True)
                    nc.scalar.activation(out=expt[(h, mt)][:, csl], in_=ps,
                                         func=AF.Exp)
                ps = ptile([16, SCH], tag="aux")
                for mt in range(3):
                    p = MT[mt]
                    nc.tensor.matmul(ps, indRT_s[0:p, mt * 16:(mt + 1) * 16],
                                     expt[(h, mt)][:, csl], start=(mt == 0),
                                     stop=(mt == 2))
                nc.scalar.copy(out=rsb[h][:, csl], in_=ps)
                ps = ptile([1, SCH], tag="aux")
                nc.tensor.matmul(ps, ones16, rsb[h][:, csl],
                                 start=True, stop=True)
                ssb = ap_.tile([1, SCH], RDT, tag="ssb", bufs=4, name="ssb")
                nc.scalar.copy(out=ssb, in_=ps)
                ps = ptile([64, SCH], tag="aux")
                nc.tensor.matmul(ps, _r(ones1r[:, 0:64]), _r(ssb),
                                 start=True, stop=True)
                nc.vector.reciprocal_approx_fast(out=msb[h][:, csl], in_=ps)
        for cc in range(NCH):
            csl = slice(cc * CH, (cc + 1) * CH)
            ps = ptile([128, CH], tag="o2", pool=po2)
            for j, h in enumerate(heads):
                osl = slice(j * 64, (j + 1) * 64)
                for mt in range(3):
                    nc.tensor.matmul(
                        ps[osl, :], vbt[mt][:, h * DH:(h + 1) * DH],
                        expt[(h, mt)][:, csl], start=(mt == 0), stop=False,
                        skip_group_check=True)
                for i in range(10):
                    n = cc * 10 + i
                    nc.tensor.matmul(
                        ps[osl, i * 40:(i + 1) * 40], vaq[h][:, :, n],
                        rsb[h][:, n * 40:(n + 1) * 40],
                        start=False, stop=(i == 9),
                        skip_group_check=True)
            for j, h in enumerate(heads):
                nc.vector.tensor_tensor(
                    out=attn[hp][j * 64:(j + 1) * 64, csl],
                    in0=ps[j * 64:(j + 1) * 64, :], in1=msb[h][:, csl],
                    op=ALU.mult)

    # ---------- P6/P7/P8 fused per chunk ----------
    upar = [ctile([128, NCH, BCn], f"upar{i}") for i in range(4)]
    ssump = ctile([1, NCH, BCn], "ssump")
    hpool = tc.alloc_tile_pool(name="hp", bufs=2)
    for cc in range(NCH):
        csl = slice(cc * CH, (cc + 1) * CH)
        hall, a1, a2 = [], [], []
        for mt in range(4):
            ps = ptile([128, CH], tag="hps")
            for k in range(4):
                nc.tensor.matmul(ps, wc[k][:, mt * 128:(mt + 1) * 128],
                                 attn[k][:, csl], start=(k == 0), stop=(k == 3))
            hl = hpool.tile([128, CH], RDT, tag=f"hall{mt}", name=f"hall{mt}")
            hall.append(hl)
            for bl in range(BPC):
                ov = _v(hl, [[BCn, 10], [1, NCv]], off=bl * NCv)
                nc.vector.scalar_tensor_tensor(
                    out=ov, in0=_v(ps, [[BCn, 10], [1, NCv]], off=bl * NCv),
                    scalar=b3c_s[:, mt:mt + 1],
                    in1=_v(a1w3[mt], [[1, 10], [0, NCv]],
                           off=cc * 10 + bl * NH),
                    op0=ALU.add, op1=ALU.add)
                nc.vector.tensor_tensor(
                    out=ov, in0=ov,
                    in1=_v(b1w3[mt], [[0, 10], [1, NCv]], off=bl * NCv),
                    op=ALU.add)
        for mt in range(4):
            ps = ptile([128, CH], tag="hps")
            mm_acc(ps, [wd1a[k][:, mt * 128:(mt + 1) * 128] for k in range(4)],
                   [hall[k] for k in range(4)])
            tmp = hpool.tile([128, CH], F32, tag="a1tmp", name="a1tmp")
            for bl in range(BPC):
                nc.vector.tensor_tensor(
                    out=_v(tmp, [[BCn, 10], [1, NCv]], off=bl * NCv),
                    in0=_v(ps, [[BCn, 10], [1, NCv]], off=bl * NCv),
                    in1=_v(cwd1[mt], [[0, 10], [1, NCv]], off=bl * NCv),
                    op=ALU.add)
            atl = hpool.tile([128, CH], BF16, tag=f"a1_{mt}", name=f"a1_{mt}")
            a1.append(atl)
            nc.scalar.activation(out=atl, in_=tmp, func=AF.Tanh,
                                 bias=bd1c_s[:, mt:mt + 1], scale=1.0)
        for mt in range(2):
            ps = ptile([128, CH], tag="hps")
            for k in range(4):
                nc.tensor.matmul(ps, wd2[k][:, mt * 128:(mt + 1) * 128],
                                 a1[k], start=(k == 0), stop=(k == 3))
            atl = hpool.tile([128, CH], BF16, tag=f"a2_{mt}", name=f"a2_{mt}")
            a2.append(atl)
            nc.scalar.activation(out=atl, in_=ps, func=AF.Tanh,
                                 bias=bd2c_s[:, mt:mt + 1], scale=1.0)
        ps = ptile([1, CH], tag="a3ps")
        for k in range(2):
            nc.tensor.matmul(ps, wd3[k], a2[k], start=(k == 0), stop=(k == 1))
        ea3 = hpool.tile([1, CH], RDT, tag="ea3", name="ea3")
        nc.scalar.activation(out=ea3, in_=ps, func=AF.Exp, bias=bd3c_s,
                             scale=1.0)
        nc.vector.tensor_reduce(
            out=ssump[:, cc, :], in_=_v(ea3, [[1, BCn], [BCn, 10]]),
            axis=AX.X, op=ALU.add)
        psw = ptile([128, CH], tag="wps")
        nc.tensor.matmul(psw, _r(ones1r), _r(ea3), start=True, stop=True)
        for mt in range(4):
            tmp = hpool.tile([128, CH], F32, tag="utmp", name="utmp")
            nc.vector.tensor_tensor(out=tmp, in0=hall[mt], in1=psw,
                                    op=ALU.mult)
            nc.vector.tensor_reduce(
                out=upar[mt][:, cc, :], in_=_v(tmp, [[1, BCn], [BCn, 10]]),
                axis=AX.X, op=ALU.add)

    ssum = ctile([1, BCn], "ssum")
    nc.vector.tensor_reduce(out=ssum, in_=_v(ssump, [[1, BCn], [BCn, NCH]]),
                            axis=AX.X, op=ALU.add)
    rw = ctile([1, BCn], "rw", RDT)
    nc.vector.reciprocal(out=rw, in_=ssum)
    psr = ptile([128, BCn], tag="rwb")
    nc.tensor.matmul(psr, _r(ones1r), _r(rw), start=True, stop=True)
    for mt in range(4):
        us = ctile([128, BCn], f"usb{mt}")
        nc.vector.tensor_reduce(
            out=us, in_=_v(upar[mt], [[1, BCn], [BCn, NCH]]),
            axis=AX.X, op=ALU.add)
        nc.vector.tensor_tensor(out=us, in0=us, in1=psr, op=ALU.mult)
        nc.sync.dma_start(out=D_["uT"][mt * 128:(mt + 1) * 128, :], in_=us)
    # release remaining pools in LIFO order per (space, side) stack
    hpool.release()
    qpool.release()
    ap_.release()
    con.release()
    pV.release()
    po2.release()
    pp.release()


def _host_prep(h, c, W1, b1, W2, b2, W3, b3, Wd1, bd1, Wd2, bd2, Wd3, bd3,
               Win, bin_, Wout, bout):
    f = np.float32
    ct = np.ascontiguousarray
    h = np.asarray(h, f); c = np.asarray(c, f)
    W1, b1, W2, b2, W3, b3 = (np.asarray(x, f) for x in (W1, b1, W2, b2, W3, b3))
    Wd1, bd1, Wd2, bd2, Wd3, bd3 = (np.asarray(x, f) for x in
                                    (Wd1, bd1, Wd2, bd2, Wd3, bd3))
    Win, bin_, Wout, bout = (np.asarray(x, f) for x in (Win, bin_, Wout, bout))

    W3a, W3b = W3[:, :512], W3[:, 512:]
    Wfa = W3a @ W1[:, :1536]
    Wfd = W3a @ W1[:, 1536:]
    b3p = b3 + W3b @ bout + W3a @ b1
    Winq, Wink, Winv = Win[:512], Win[512:1024], Win[1024:]
    binq = (bin_[:512] + Winq @ b2) / 8.0
    bink = bin_[512:1024] + Wink @ b2
    binv = bin_[1024:] + Winv @ b2
    Wc = W3b @ Wout

    hT = ct(h.transpose(2, 0, 1).reshape(512, HTOK))
    hlT = np.roll(h, 1, axis=1).transpose(2, 0, 1).reshape(512, HTOK)
    hrT = np.roll(h, -1, axis=1).transpose(2, 0, 1).reshape(512, HTOK)
    hcat_all = np.concatenate([hlT, hT, hrT], axis=0)
    cT_all = ct(c.transpose(2, 0, 1).reshape(512, CTOK))

    col = lambda v, k: ct(np.asarray(v, f).reshape(k, 128).T)
    indM = np.zeros((16, CTOK), BF)
    for m in range(CTOK):
        indM[m // NCv, m] = 1
    indRT = np.zeros((128, 3, 16), BF)
    for m in range(CTOK):
        indRT[m % 128, m // 128, m // NCv] = 1

    shared = dict(
        hT_all=hT, cT_all=cT_all,
        w2aT=ct(W2[:, :512].T), w2bT=ct(W2[:, 512:].T),
        winqT=ct(Winq.T / 8.0), winkT=ct(Wink.T), winvT=ct(Winv.T),
        binq=col(binq, 4), bink=col(bink, 4),
        binv=ct(binv.reshape(1, 512)),
        wfaT=ct(Wfa.T), wfdT=ct(Wfd.T), b3c=col(b3p, 4),
        wcT=ct(Wc.T.astype(BF)),
        wd1aT=ct(Wd1[:, :512].T), wd1bT=ct(Wd1[:, 512:].T),
        bd1c=col(bd1, 4),
        wd2T=ct(Wd2.T.astype(BF)), bd2c=col(bd2, 2),
        wd3T=ct(Wd3.T.astype(BF)),
        bd3c=np.asarray(bd3, f).reshape(1, 1),
        indM=indM, indRT=ct(indRT.reshape(128, 48)),
    )
    in_maps = []
    for r in range(NCORES):
        m = dict(shared)
        m["hcat_own"] = ct(hcat_all[:, 2 * r * NH:(2 * r + 2) * NH])
        m["cT_own"] = ct(cT_all[:, 2 * r * NCv:(2 * r + 2) * NCv])
        in_maps.append(m)
    return in_maps


_PROG = None


def _get_prog():
    global _PROG
    if _PROG is None:
        _PROG = _build()
    return _PROG


def kernel(**inputs):
    nc = _get_prog()
    in_maps = _host_prep(**inputs)
    res = bass_utils.run_bass_kernel_spmd(nc, in_maps,
                                          core_ids=list(range(NCORES)))
    u = np.empty((B, NCv, 512), np.float32)
    for r in range(NCORES):
        uT = res.results[r]["uT"]
        u[BPC * r:BPC * (r + 1)] = uT.T.reshape(BPC, NCv, 512)
    return u
